# revision 2
# baseline (speedup 1.0000x reference)
"""Trainium2 Bass kernel for nn_MappingNetwork (histogram_binning).

reference: seeds = searchsorted(linspace(-1e5, 1e5, 1e8, f32), z[:, 0], 'left');
           out = broadcast(seeds[:, None], (16384, 512)).astype(int32)

The buckets are uniform with spacing 2e5/(1e8-1) ~= 0.002, so the searchsorted
index admits the closed-form affine  seed = (z + 1e5) * 500.  Computed in f32
(half-ULP rounding at 1e5 on the add; the product lands on exact multiples of
4 at magnitude 5e7, so the int32 convert is exact), this sits within 6 indices
of the bit-exact searchsorted result — validated against an exact f64-fma
emulation of the XLA-CPU linspace over the full data range.  Relative error
~1.2e-7 against seeds of magnitude 5e7, five orders of magnitude inside the
2e-2 gate.

Per core (2048 rows, row r = p*16 + n -> zv[p, n]):
  - load z[:, 0] shard (8KB) into [128, 16] SBUF,
  - one fused DVE tensor_scalar per query column computes the affine AND
    broadcasts it 128 wide with int32 convert-on-write (127ns each, eager so
    the DMA streams start as soon as their first query is ready),
  - THREE concurrent DMA streams (SP + Activation HWDGE queues, Pool SWDGE)
    each write ~1/3 of the output; every DMA amplifies its 512B/partition
    source block 4x via a stride-0 source AP, writing 2KB/partition.
  - all DMAs signal per-stream completion semaphores which each engine waits
    on before the end-of-block barrier (completion-safe on hardware).

Queries are round-robined across the streams in DVE production order, with
query 15 split 3+1 between the SP and Act streams to balance stream end
times against their staggered starts.
"""

import numpy as np

N_CORES = 8
B = 16384
W = 512
ROWS = B // N_CORES  # 2048 rows per core
P = 128
NQ = ROWS // P  # 16 queries per partition
REP = 128  # SBUF replication width; DMA repeats each 512B block 4x
NB = W // REP  # 4 blocks per query

# per-stream DMA lists: (query, first_block, n_blocks)
ASSIGN = {
    "sync": [(q, 0, 4) for q in range(0, 15, 3)] + [(15, 0, 3)],
    "scalar": [(q, 0, 4) for q in range(1, 15, 3)] + [(15, 3, 1)],
    "gpsimd": [(q, 0, 4) for q in range(2, 15, 3)],
}

_nc_cache = {}


def build_nc():
    if "nc" in _nc_cache:
        return _nc_cache["nc"]
    import concourse.bass as bass
    import concourse.mybir as mybir

    dt = mybir.dt
    alu = mybir.AluOpType

    nc = bass.Bass(detect_race_conditions=False)
    zcol = nc.dram_tensor("zcol", [ROWS, 1], dt.float32, kind="ExternalInput")
    out = nc.dram_tensor("out", [ROWS, W], dt.int32, kind="ExternalOutput")

    # row r = p*NQ + n  ->  zv[p, n]
    zsrc = zcol.rearrange("(p n) one -> p (n one)", p=P)
    odst = out.rearrange("(p n) w -> p n w", p=P)

    v = nc.vector

    from contextlib import ExitStack

    es = ExitStack()
    with es:
        zv = es.enter_context(nc.sbuf_tensor("zv", [P, NQ], dt.float32))
        rep = es.enter_context(nc.sbuf_tensor("rep", [P, NQ * REP], dt.int32))
        in_sem = es.enter_context(nc.semaphore("in_sem"))
        dve_sem = es.enter_context(nc.semaphore("dve_sem"))
        osems = {
            name: es.enter_context(nc.semaphore(f"osem_{name}")) for name in ASSIGN
        }
        block = es.enter_context(nc.Block())

        def make_body(name, dmas):
            def body(eng):
                if name == "sync":
                    eng.dma_start(out=zv[:, :], in_=zsrc).then_inc(in_sem, 16)
                for q, b0, nb in dmas:
                    eng.wait_ge(dve_sem, q + 1)
                    rsrc = (
                        rep[:, q * REP : (q + 1) * REP]
                        .unsqueeze(1)
                        .broadcast_to([P, NB, REP])[:, b0 : b0 + nb, :]
                    )
                    rdst = odst[:, q, b0 * REP : (b0 + nb) * REP].rearrange(
                        "p (b r) -> p b r", r=REP
                    )
                    eng.dma_start(out=rdst, in_=rsrc).then_inc(osems[name], 16)
                eng.wait_ge(osems[name], 16 * len(dmas))

            return body

        for name, dmas in ASSIGN.items():
            getattr(block, name)(make_body(name, dmas))

        @block.vector
        def _(vector):
            vector.wait_ge(in_sem, 16)
            for n in range(NQ):
                # rep[p, n, r] = int32((zv[p, n] + 1e5) * 500), broadcast over r
                v.tensor_scalar(
                    rep[:, n * REP : (n + 1) * REP].rearrange(
                        "p (n r) -> p n r", r=REP
                    ),
                    zv[:, n : n + 1].unsqueeze(-1).broadcast_to([P, 1, REP]),
                    100000.0,
                    500.0,
                    alu.add,
                    alu.mult,
                ).then_inc(dve_sem, 1)

    _nc_cache["nc"] = nc
    return nc


def kernel(z, c=None, **_unused):
    z = np.ascontiguousarray(np.asarray(z), dtype=np.float32)
    assert z.shape == (B, W), z.shape
    nc = build_nc()
    from concourse.bass_utils import run_bass_kernel_spmd

    in_maps = []
    for i in range(N_CORES):
        zc = np.ascontiguousarray(z[i * ROWS : (i + 1) * ROWS, 0:1])
        in_maps.append({"zcol": zc})
    res = run_bass_kernel_spmd(nc, in_maps, core_ids=list(range(N_CORES)))
    globals()["LAST_RESULT"] = res
    return np.concatenate([r["out"] for r in res.results], axis=0).astype(np.int32)


# revision 5
# speedup vs baseline: 2.1331x; 2.1331x over previous
"""Trainium2 Bass kernel for nn_MappingNetwork (histogram_binning).

reference: seeds = searchsorted(linspace(-1e5, 1e5, 1e8, f32), z[:, 0], 'left');
           out = broadcast(seeds[:, None], (16384, 512)).astype(int32)

The buckets are uniform with spacing 2e5/(1e8-1) ~= 0.002, so the searchsorted
index admits the closed-form affine  seed = (z + 1e5) * 500  (the spec's
sharding_hint itself suggests eliminating the bucket buffer via this closed
form).  Computed in f32 it sits within 6 indices of the bit-exact searchsorted
result — validated against an exact f64-fma emulation of the XLA-CPU linspace
over the full data range.  Relative error ~1.2e-7 against seeds of magnitude
5e7, five orders of magnitude inside the 2e-2 gate.

Per core (2048 rows, row r = p*16 + n -> zv[p, n]):
  1. sync engine loads z[:, 0] shard (8KB) into [128, 16] SBUF, waits on its
     own DMA-completion semaphore, and relays readiness to the DVE through a
     plain engine semaphore (a correct completion -> relay -> consumer chain
     on hardware, far lower latency than a cross-engine DMA-semaphore wait),
  2. one DVE tensor_scalar computes all 2048 seeds, each duplicated into a
     width-2 pair, with int32 convert-on-write ([128, 16, 2], ~100ns),
  3. sync stages the seed pairs (16KB) to an internal DRAM scratch [2048, 2],
     waits for it to land, and relays to the scalar engine,
  4. two DRAM->DRAM broadcast DMAs (sync: columns 0-255, scalar engine:
     columns 256-511, running concurrently) fan each row's 8B pair out 128x
     into the 4MB output shard: src keeps the stride-0 repeat dim in the
     middle with a contiguous innermost pair (DGE-legal), dst walks
     [row, repeat, pair] with every ISA dim count under 2^16.
Both engines hold the end-of-block barrier on their own DMA-completion
semaphores, so the kernel never signals done before the output is in DRAM.

Sharding: batch 16384 -> 8 cores x 2048 rows.
"""

import numpy as np

N_CORES = 8
B = 16384
W = 512
ROWS = B // N_CORES  # 2048 rows per core
P = 128
NQ = ROWS // P  # 16 queries per partition
CW = 2  # staged copy width (seed pair); innermost contiguous run of the d2d
HALF = W // 2  # columns per broadcast DMA
NJ = HALF // CW  # repeat count per broadcast DMA (128)

_nc_cache = {}


def build_nc():
    if "nc" in _nc_cache:
        return _nc_cache["nc"]
    import concourse.bass as bass
    import concourse.mybir as mybir

    dt = mybir.dt
    alu = mybir.AluOpType

    nc = bass.Bass(detect_race_conditions=False)
    zcol = nc.dram_tensor("zcol", [ROWS, 1], dt.float32, kind="ExternalInput")
    out = nc.dram_tensor("out", [ROWS, W], dt.int32, kind="ExternalOutput")
    stage = nc.dram_tensor("stage", [ROWS, CW], dt.int32, kind="Internal")

    zsrc = zcol.rearrange("(p n) one -> p (n one)", p=P)

    v = nc.vector

    from contextlib import ExitStack

    es = ExitStack()
    with es:
        zv = es.enter_context(nc.sbuf_tensor("zv", [P, NQ], dt.float32))
        seeds = es.enter_context(nc.sbuf_tensor("seeds", [P, NQ * CW], dt.int32))
        in_sem = es.enter_context(nc.semaphore("in_sem"))
        relay_sem = es.enter_context(nc.semaphore("relay_sem"))
        dve_sem = es.enter_context(nc.semaphore("dve_sem"))
        st_sem = es.enter_context(nc.semaphore("st_sem"))
        st_relay = es.enter_context(nc.semaphore("st_relay"))
        d2d_sem = es.enter_context(nc.semaphore("d2d_sem"))
        d2d_sem2 = es.enter_context(nc.semaphore("d2d_sem2"))
        block = es.enter_context(nc.Block())

        def d2d(eng, col0, sem):
            # out[r, col0 + j*2 + c] = stage[r, c]
            d2d_out = out[:, col0 : col0 + HALF].rearrange(
                "r (j c) -> r j c", c=CW
            )
            d2d_in = stage[:, :].unsqueeze(1).broadcast_to([ROWS, NJ, CW])
            eng.dma_start(out=d2d_out, in_=d2d_in).then_inc(sem, 16)

        @block.sync
        def _(sync):
            sync.dma_start(out=zv[:, :], in_=zsrc).then_inc(in_sem, 16)
            sync.wait_ge(in_sem, 16)  # true DMA completion (same engine)
            sync.sem_inc(relay_sem, 1)  # relay readiness to DVE
            sync.wait_ge(dve_sem, 1)
            # stage[p*16+n, c] = seeds[p, (n c)]
            sync.dma_start(
                out=stage.rearrange("(p n) c -> p (n c)", p=P),
                in_=seeds[:, :],
            ).then_inc(st_sem, 16)
            sync.wait_ge(st_sem, 16)  # staging landed (same engine)
            sync.sem_inc(st_relay, 1)  # relay to scalar engine
            d2d(sync, 0, d2d_sem)
            sync.wait_ge(d2d_sem, 16)  # left half landed

        @block.scalar
        def _(scalar):
            scalar.wait_ge(st_relay, 1)
            d2d(scalar, HALF, d2d_sem2)
            scalar.wait_ge(d2d_sem2, 16)  # right half landed

        @block.vector
        def _(vector):
            vector.wait_ge(relay_sem, 1)
            # seeds[p, n, c] = int32((zv[p, n] + 1e5) * 500) for c in {0, 1}
            v.tensor_scalar(
                seeds[:, :].rearrange("p (n c) -> p n c", c=CW),
                zv[:, :].unsqueeze(-1).broadcast_to([P, NQ, CW]),
                100000.0,
                500.0,
                alu.add,
                alu.mult,
            ).then_inc(dve_sem, 1)

    _nc_cache["nc"] = nc
    return nc


def kernel(z, c=None, **_unused):
    z = np.ascontiguousarray(np.asarray(z), dtype=np.float32)
    assert z.shape == (B, W), z.shape
    nc = build_nc()
    from concourse.bass_utils import run_bass_kernel_spmd

    in_maps = []
    for i in range(N_CORES):
        zc = np.ascontiguousarray(z[i * ROWS : (i + 1) * ROWS, 0:1])
        in_maps.append({"zcol": zc})
    res = run_bass_kernel_spmd(nc, in_maps, core_ids=list(range(N_CORES)))
    globals()["LAST_RESULT"] = res
    return np.concatenate([r["out"] for r in res.results], axis=0).astype(np.int32)


# revision 8
# speedup vs baseline: 2.1840x; 1.0239x over previous
"""Trainium2 Bass kernel for nn_MappingNetwork (histogram_binning).

reference: seeds = searchsorted(linspace(-1e5, 1e5, 1e8, f32), z[:, 0], 'left');
           out = broadcast(seeds[:, None], (16384, 512)).astype(int32)

The buckets are uniform with spacing 2e5/(1e8-1) ~= 0.002, so the searchsorted
index admits the closed-form affine  seed = (z + 1e5) * 500  (the spec's
sharding_hint itself suggests eliminating the bucket buffer via this closed
form).  Computed in f32 it sits within 6 indices of the bit-exact searchsorted
result — validated against an exact f64-fma emulation of the XLA-CPU linspace
over the full data range.  Relative error ~1.2e-7 against seeds of magnitude
5e7, five orders of magnitude inside the 2e-2 gate.

Per core (2048 rows, row r = p*16 + n -> zv[p, n]):
  1. sync engine loads z[:, 0] shard (8KB) into [128, 16] SBUF, waits on its
     own DMA-completion semaphore, and relays readiness to the DVE through a
     plain engine semaphore (a correct completion -> relay -> consumer chain
     on hardware, far lower latency than a cross-engine DMA-semaphore wait),
  2. one DVE tensor_scalar computes all 2048 seeds, each duplicated into a
     width-2 pair, with int32 convert-on-write ([128, 16, 2], ~100ns),
  3. the sync and scalar engines INDEPENDENTLY stage the seed pairs (16KB)
     to their own internal DRAM scratch, wait on their own completion
     semaphores (no cross-engine staging handoff),
  4. two DRAM->DRAM broadcast DMAs (sync: columns 0-255, scalar engine:
     columns 256-511, running concurrently) fan each row's 8B pair out 128x
     into the 4MB output shard: src keeps the stride-0 repeat dim in the
     middle with a contiguous innermost pair (DGE-legal), dst walks
     [row, repeat, pair] with every ISA dim count under 2^16.
Both engines hold the end-of-block barrier on their own DMA-completion
semaphores, so the kernel never signals done before the output is in DRAM.

Sharding: batch 16384 -> 8 cores x 2048 rows.
"""

import numpy as np

N_CORES = 8
B = 16384
W = 512
ROWS = B // N_CORES  # 2048 rows per core
P = 128
NQ = ROWS // P  # 16 queries per partition
CW = 2  # staged copy width (seed pair); innermost contiguous run of the d2d
HALF = W // 2  # columns per broadcast DMA
NJ = HALF // CW  # repeat count per broadcast DMA (128)

_nc_cache = {}


def build_nc():
    if "nc" in _nc_cache:
        return _nc_cache["nc"]
    import concourse.bass as bass
    import concourse.mybir as mybir

    dt = mybir.dt
    alu = mybir.AluOpType

    nc = bass.Bass(detect_race_conditions=False)
    zcol = nc.dram_tensor("zcol", [ROWS, 1], dt.float32, kind="ExternalInput")
    out = nc.dram_tensor("out", [ROWS, W], dt.int32, kind="ExternalOutput")
    stage_a = nc.dram_tensor("stage_a", [ROWS, CW], dt.int32, kind="Internal")
    stage_b = nc.dram_tensor("stage_b", [ROWS, CW], dt.int32, kind="Internal")

    zsrc = zcol.rearrange("(p n) one -> p (n one)", p=P)

    v = nc.vector

    from contextlib import ExitStack

    es = ExitStack()
    with es:
        zv = es.enter_context(nc.sbuf_tensor("zv", [P, NQ], dt.float32))
        seeds = es.enter_context(nc.sbuf_tensor("seeds", [P, NQ * CW], dt.int32))
        in_sem = es.enter_context(nc.semaphore("in_sem"))
        relay_sem = es.enter_context(nc.semaphore("relay_sem"))
        dve_sem = es.enter_context(nc.semaphore("dve_sem"))
        st_sem_a = es.enter_context(nc.semaphore("st_sem_a"))
        st_sem_b = es.enter_context(nc.semaphore("st_sem_b"))
        d2d_sem = es.enter_context(nc.semaphore("d2d_sem"))
        d2d_sem2 = es.enter_context(nc.semaphore("d2d_sem2"))
        block = es.enter_context(nc.Block())

        def chain(eng, stage, col0, st_sem, sem):
            # stage[p*16+n, c] = seeds[p, (n c)]
            eng.dma_start(
                out=stage.rearrange("(p n) c -> p (n c)", p=P),
                in_=seeds[:, :],
            ).then_inc(st_sem, 16)
            eng.wait_ge(st_sem, 16)  # staging landed (same engine)
            # out[r, col0 + j*2 + c] = stage[r, c]
            d2d_out = out[:, col0 : col0 + HALF].rearrange(
                "r (j c) -> r j c", c=CW
            )
            d2d_in = stage[:, :].unsqueeze(1).broadcast_to([ROWS, NJ, CW])
            eng.dma_start(out=d2d_out, in_=d2d_in).then_inc(sem, 16)
            eng.wait_ge(sem, 16)  # output half landed before kernel end

        @block.sync
        def _(sync):
            sync.dma_start(out=zv[:, :], in_=zsrc).then_inc(in_sem, 16)
            sync.wait_ge(in_sem, 16)  # true DMA completion (same engine)
            sync.sem_inc(relay_sem, 1)  # relay readiness to DVE
            sync.wait_ge(dve_sem, 1)
            chain(sync, stage_a, 0, st_sem_a, d2d_sem)

        @block.scalar
        def _(scalar):
            scalar.wait_ge(dve_sem, 1)
            chain(scalar, stage_b, HALF, st_sem_b, d2d_sem2)

        @block.vector
        def _(vector):
            vector.wait_ge(relay_sem, 1)
            # seeds[p, n, c] = int32((zv[p, n] + 1e5) * 500) for c in {0, 1}
            v.tensor_scalar(
                seeds[:, :].rearrange("p (n c) -> p n c", c=CW),
                zv[:, :].unsqueeze(-1).broadcast_to([P, NQ, CW]),
                100000.0,
                500.0,
                alu.add,
                alu.mult,
            ).then_inc(dve_sem, 1)

    _nc_cache["nc"] = nc
    return nc


def kernel(z, c=None, **_unused):
    z = np.ascontiguousarray(np.asarray(z), dtype=np.float32)
    assert z.shape == (B, W), z.shape
    nc = build_nc()
    from concourse.bass_utils import run_bass_kernel_spmd

    in_maps = []
    for i in range(N_CORES):
        zc = np.ascontiguousarray(z[i * ROWS : (i + 1) * ROWS, 0:1])
        in_maps.append({"zcol": zc})
    res = run_bass_kernel_spmd(nc, in_maps, core_ids=list(range(N_CORES)))
    globals()["LAST_RESULT"] = res
    return np.concatenate([r["out"] for r in res.results], axis=0).astype(np.int32)


# revision 9
# speedup vs baseline: 2.3228x; 1.0635x over previous
"""Trainium2 Bass kernel for nn_MappingNetwork (histogram_binning).

reference: seeds = searchsorted(linspace(-1e5, 1e5, 1e8, f32), z[:, 0], 'left');
           out = broadcast(seeds[:, None], (16384, 512)).astype(int32)

The buckets are uniform with spacing 2e5/(1e8-1) ~= 0.002, so the searchsorted
index admits the closed-form affine  seed = (z + 1e5) * 500  (the spec's
sharding_hint itself suggests eliminating the bucket buffer via this closed
form).  Computed in f32 it sits within 6 indices of the bit-exact searchsorted
result — validated against an exact f64-fma emulation of the XLA-CPU linspace
over the full data range.  Relative error ~1.2e-7 against seeds of magnitude
5e7, five orders of magnitude inside the 2e-2 gate.

Per core (2048 rows, row r = p*16 + n -> zv[p, n]):
  1. the Pool engine loads the z[:, 0] shard (8KB) into [128, 16] SBUF via
     its software-DGE queue, waits on its own DMA-completion semaphore, and
     computes all 2048 seeds itself (one Pool-engine tensor_scalar, each
     seed duplicated into a width-2 pair with int32 convert-on-write) — a
     single-engine load->compute chain with no cross-engine DMA-semaphore
     latency,
  2. the sync and scalar engines each wait on the compute semaphore and
     INDEPENDENTLY stage the seed pairs (16KB) to their own internal DRAM
     scratch, waiting on their own completion semaphores,
  3. two DRAM->DRAM broadcast DMAs (sync: columns 0-255, scalar engine:
     columns 256-511, running concurrently) fan each row's 8B pair out 128x
     into the 4MB output shard: src keeps the stride-0 repeat dim in the
     middle with a contiguous innermost pair (DGE-legal), dst walks
     [row, repeat, pair] with every ISA dim count under 2^16.
Every engine holds the end-of-block barrier on its own DMA-completion
semaphores, so the kernel never signals done before the output is in DRAM.

Sharding: batch 16384 -> 8 cores x 2048 rows.
"""

import numpy as np

N_CORES = 8
B = 16384
W = 512
ROWS = B // N_CORES  # 2048 rows per core
P = 128
NQ = ROWS // P  # 16 queries per partition
CW = 2  # staged copy width (seed pair); innermost contiguous run of the d2d
HALF = W // 2  # columns per broadcast DMA
NJ = HALF // CW  # repeat count per broadcast DMA (128)

_nc_cache = {}


def build_nc():
    if "nc" in _nc_cache:
        return _nc_cache["nc"]
    import concourse.bass as bass
    import concourse.mybir as mybir

    dt = mybir.dt
    alu = mybir.AluOpType

    nc = bass.Bass(detect_race_conditions=False)
    zcol = nc.dram_tensor("zcol", [ROWS, 1], dt.float32, kind="ExternalInput")
    out = nc.dram_tensor("out", [ROWS, W], dt.int32, kind="ExternalOutput")
    stage_a = nc.dram_tensor("stage_a", [ROWS, CW], dt.int32, kind="Internal")
    stage_b = nc.dram_tensor("stage_b", [ROWS, CW], dt.int32, kind="Internal")

    zsrc = zcol.rearrange("(p n) one -> p (n one)", p=P)

    from contextlib import ExitStack

    es = ExitStack()
    with es:
        zv = es.enter_context(nc.sbuf_tensor("zv", [P, NQ], dt.float32))
        seeds = es.enter_context(nc.sbuf_tensor("seeds", [P, NQ * CW], dt.int32))
        in_sem = es.enter_context(nc.semaphore("in_sem"))
        cmp_sem = es.enter_context(nc.semaphore("cmp_sem"))
        st_sem_a = es.enter_context(nc.semaphore("st_sem_a"))
        st_sem_b = es.enter_context(nc.semaphore("st_sem_b"))
        d2d_sem = es.enter_context(nc.semaphore("d2d_sem"))
        d2d_sem2 = es.enter_context(nc.semaphore("d2d_sem2"))
        block = es.enter_context(nc.Block())

        @block.gpsimd
        def _(g):
            g.dma_start(out=zv[:, :], in_=zsrc).then_inc(in_sem, 16)
            g.wait_ge(in_sem, 16)  # true DMA completion (same engine)
            # seeds[p, n, c] = int32((zv[p, n] + 1e5) * 500) for c in {0, 1}
            nc.gpsimd.tensor_scalar(
                seeds[:, :].rearrange("p (n c) -> p n c", c=CW),
                zv[:, :].unsqueeze(-1).broadcast_to([P, NQ, CW]),
                100000.0,
                500.0,
                alu.add,
                alu.mult,
            ).then_inc(cmp_sem, 1)

        def chain(eng, stage, col0, st_sem, sem):
            eng.wait_ge(cmp_sem, 1)
            # stage[p*16+n, c] = seeds[p, (n c)]
            eng.dma_start(
                out=stage.rearrange("(p n) c -> p (n c)", p=P),
                in_=seeds[:, :],
            ).then_inc(st_sem, 16)
            eng.wait_ge(st_sem, 16)  # staging landed (same engine)
            # out[r, col0 + j*2 + c] = stage[r, c]
            d2d_out = out[:, col0 : col0 + HALF].rearrange(
                "r (j c) -> r j c", c=CW
            )
            d2d_in = stage[:, :].unsqueeze(1).broadcast_to([ROWS, NJ, CW])
            eng.dma_start(out=d2d_out, in_=d2d_in).then_inc(sem, 16)
            eng.wait_ge(sem, 16)  # output half landed before kernel end

        @block.sync
        def _(sync):
            chain(sync, stage_a, 0, st_sem_a, d2d_sem)

        @block.scalar
        def _(scalar):
            chain(scalar, stage_b, HALF, st_sem_b, d2d_sem2)

    _nc_cache["nc"] = nc
    return nc


def kernel(z, c=None, **_unused):
    z = np.ascontiguousarray(np.asarray(z), dtype=np.float32)
    assert z.shape == (B, W), z.shape
    nc = build_nc()
    from concourse.bass_utils import run_bass_kernel_spmd

    in_maps = []
    for i in range(N_CORES):
        zc = np.ascontiguousarray(z[i * ROWS : (i + 1) * ROWS, 0:1])
        in_maps.append({"zcol": zc})
    res = run_bass_kernel_spmd(nc, in_maps, core_ids=list(range(N_CORES)))
    globals()["LAST_RESULT"] = res
    return np.concatenate([r["out"] for r in res.results], axis=0).astype(np.int32)


# revision 10
# speedup vs baseline: 2.3341x; 1.0049x over previous
"""Trainium2 Bass kernel for nn_MappingNetwork (histogram_binning).

reference: seeds = searchsorted(linspace(-1e5, 1e5, 1e8, f32), z[:, 0], 'left');
           out = broadcast(seeds[:, None], (16384, 512)).astype(int32)

The buckets are uniform with spacing 2e5/(1e8-1) ~= 0.002, so the searchsorted
index admits the closed-form affine  seed = (z + 1e5) * 500  (the spec's
sharding_hint itself suggests eliminating the bucket buffer via this closed
form).  Computed in f32 it sits within 6 indices of the bit-exact searchsorted
result — validated against an exact f64-fma emulation of the XLA-CPU linspace
over the full data range.  Relative error ~1.2e-7 against seeds of magnitude
5e7, five orders of magnitude inside the 2e-2 gate.

Per core (2048 rows, row r = p*16 + n -> zv[p, n]):
  1. the Pool engine loads the z[:, 0] shard (8KB) into [128, 16] SBUF via
     its software-DGE queue, waits on its own DMA-completion semaphore, and
     computes all 2048 seeds itself (one Pool-engine tensor_scalar, each
     seed duplicated into a width-2 pair with int32 convert-on-write) — a
     single-engine load->compute chain with no cross-engine DMA-semaphore
     latency,
  2. all three DMA-capable engines (sync, scalar, Pool) INDEPENDENTLY stage
     the seed pairs (16KB) to their own internal DRAM scratch and wait on
     their own completion semaphores,
  3. three concurrent DRAM->DRAM broadcast DMAs fan each row's 8B pair out
     into the 4MB output shard (sync: 250 columns, scalar: 248, Pool: 14 —
     sized so the Pool slice stays under the software-DGE 16K-descriptor
     ring limit while shaving the HWDGE halves below their bandwidth knee):
     src keeps the stride-0 repeat dim in the middle with a contiguous
     innermost pair (DGE-legal), dst walks [row, repeat, pair] with every
     ISA dim count under 2^16.
Every engine holds the end-of-block barrier on its own DMA-completion
semaphores, so the kernel never signals done before the output is in DRAM.

Sharding: batch 16384 -> 8 cores x 2048 rows.
"""

import numpy as np

N_CORES = 8
B = 16384
W = 512
ROWS = B // N_CORES  # 2048 rows per core
P = 128
NQ = ROWS // P  # 16 queries per partition
CW = 2  # staged copy width (seed pair); innermost contiguous run of the d2d

# (engine, first column, n columns); Pool slice sized for the SWDGE desc limit
SPLITS = [("sync", 0, 250), ("scalar", 250, 248), ("gpsimd", 498, 14)]

_nc_cache = {}


def build_nc():
    if "nc" in _nc_cache:
        return _nc_cache["nc"]
    import concourse.bass as bass
    import concourse.mybir as mybir

    dt = mybir.dt
    alu = mybir.AluOpType

    nc = bass.Bass(detect_race_conditions=False)
    zcol = nc.dram_tensor("zcol", [ROWS, 1], dt.float32, kind="ExternalInput")
    out = nc.dram_tensor("out", [ROWS, W], dt.int32, kind="ExternalOutput")
    stages = {
        name: nc.dram_tensor(f"stage_{name}", [ROWS, CW], dt.int32, kind="Internal")
        for name, _, _ in SPLITS
    }

    zsrc = zcol.rearrange("(p n) one -> p (n one)", p=P)

    from contextlib import ExitStack

    es = ExitStack()
    with es:
        zv = es.enter_context(nc.sbuf_tensor("zv", [P, NQ], dt.float32))
        seeds = es.enter_context(nc.sbuf_tensor("seeds", [P, NQ * CW], dt.int32))
        in_sem = es.enter_context(nc.semaphore("in_sem"))
        cmp_sem = es.enter_context(nc.semaphore("cmp_sem"))
        sems = {
            name: (
                es.enter_context(nc.semaphore(f"st_{name}")),
                es.enter_context(nc.semaphore(f"dd_{name}")),
            )
            for name, _, _ in SPLITS
        }
        block = es.enter_context(nc.Block())

        def chain(eng, name, col0, ncols, wait_cmp=True):
            st, dd = sems[name]
            stage = stages[name]
            if wait_cmp:
                eng.wait_ge(cmp_sem, 1)
            # stage[p*16+n, c] = seeds[p, (n c)]
            eng.dma_start(
                out=stage.rearrange("(p n) c -> p (n c)", p=P),
                in_=seeds[:, :],
            ).then_inc(st, 16)
            eng.wait_ge(st, 16)  # staging landed (same engine)
            # out[r, col0 + j*2 + c] = stage[r, c]
            nj = ncols // CW
            d2d_out = out[:, col0 : col0 + ncols].rearrange(
                "r (j c) -> r j c", c=CW
            )
            d2d_in = stage[:, :].unsqueeze(1).broadcast_to([ROWS, nj, CW])
            eng.dma_start(out=d2d_out, in_=d2d_in).then_inc(dd, 16)
            eng.wait_ge(dd, 16)  # output slice landed before kernel end

        @block.gpsimd
        def _(g):
            g.dma_start(out=zv[:, :], in_=zsrc).then_inc(in_sem, 16)
            g.wait_ge(in_sem, 16)  # true DMA completion (same engine)
            # seeds[p, n, c] = int32((zv[p, n] + 1e5) * 500) for c in {0, 1}
            nc.gpsimd.tensor_scalar(
                seeds[:, :].rearrange("p (n c) -> p n c", c=CW),
                zv[:, :].unsqueeze(-1).broadcast_to([P, NQ, CW]),
                100000.0,
                500.0,
                alu.add,
                alu.mult,
            ).then_inc(cmp_sem, 1)
            # same engine: seeds are ready in program order
            chain(g, "gpsimd", 498, 14, wait_cmp=False)

        @block.sync
        def _(sync):
            chain(sync, "sync", 0, 250)

        @block.scalar
        def _(scalar):
            chain(scalar, "scalar", 250, 248)

    _nc_cache["nc"] = nc
    return nc


def kernel(z, c=None, **_unused):
    z = np.ascontiguousarray(np.asarray(z), dtype=np.float32)
    assert z.shape == (B, W), z.shape
    nc = build_nc()
    from concourse.bass_utils import run_bass_kernel_spmd

    in_maps = []
    for i in range(N_CORES):
        zc = np.ascontiguousarray(z[i * ROWS : (i + 1) * ROWS, 0:1])
        in_maps.append({"zcol": zc})
    res = run_bass_kernel_spmd(nc, in_maps, core_ids=list(range(N_CORES)))
    globals()["LAST_RESULT"] = res
    return np.concatenate([r["out"] for r in res.results], axis=0).astype(np.int32)


# revision 11
# speedup vs baseline: 2.3754x; 1.0177x over previous
"""Trainium2 Bass kernel for nn_MappingNetwork (histogram_binning).

reference: seeds = searchsorted(linspace(-1e5, 1e5, 1e8, f32), z[:, 0], 'left');
           out = broadcast(seeds[:, None], (16384, 512)).astype(int32)

The buckets are uniform with spacing 2e5/(1e8-1) ~= 0.002, so the searchsorted
index admits the closed-form affine  seed = (z + 1e5) * 500  (the spec's
sharding_hint itself suggests eliminating the bucket buffer via this closed
form).  Computed in f32 it sits within 6 indices of the bit-exact searchsorted
result — validated against an exact f64-fma emulation of the XLA-CPU linspace
over the full data range.  Relative error ~1.2e-7 against seeds of magnitude
5e7, five orders of magnitude inside the 2e-2 gate.

Per core (2048 rows, row r = p*16 + n -> zv[p, n]):
  1. the Pool engine loads the z[:, 0] shard (8KB) into [128, 16] SBUF via
     its software-DGE queue, waits on its own DMA-completion semaphore, and
     computes the seeds itself: one Pool-engine tensor_scalar producing
     width-2 seed pairs (signalled to sync/scalar immediately) and a second
     producing width-8 runs for its own slice — a single-engine
     load->compute chain with no cross-engine DMA-semaphore latency,
  2. all three DMA-capable engines INDEPENDENTLY stage their seed runs
     (16KB / 64KB) to their own internal DRAM scratch and wait on their own
     completion semaphores,
  3. three concurrent DRAM->DRAM broadcast DMAs fan each row's seed run out
     into the 4MB output shard (sync: columns 0-227, scalar: 228-455, Pool:
     456-511 — the Pool slice maximized under the software-DGE
     16K-descriptor ring limit at repeat count 7): src keeps the stride-0
     repeat dim in the middle with a contiguous innermost run (DGE-legal),
     dst walks [row, repeat, run] with every ISA dim count under 2^16.
Every engine holds the end-of-block barrier on its own DMA-completion
semaphores, so the kernel never signals done before the output is in DRAM.

Sharding: batch 16384 -> 8 cores x 2048 rows.
"""

import numpy as np

N_CORES = 8
B = 16384
W = 512
ROWS = B // N_CORES  # 2048 rows per core
P = 128
NQ = ROWS // P  # 16 queries per partition
CW = 2  # sync/scalar staged run width (seed pair)
PW = 8  # Pool staged run width
POOL_COLS = 56  # Pool d2d slice: repeat count 7 -> 2048*7 descs < 16384
HALF = (W - POOL_COLS) // 2  # 228 columns each for sync/scalar

# (engine, first column, n columns, run width)
SPLITS = [
    ("sync", 0, HALF, CW),
    ("scalar", HALF, HALF, CW),
    ("gpsimd", 2 * HALF, POOL_COLS, PW),
]

_nc_cache = {}


def build_nc():
    if "nc" in _nc_cache:
        return _nc_cache["nc"]
    import concourse.bass as bass
    import concourse.mybir as mybir

    dt = mybir.dt
    alu = mybir.AluOpType

    nc = bass.Bass(detect_race_conditions=False)
    zcol = nc.dram_tensor("zcol", [ROWS, 1], dt.float32, kind="ExternalInput")
    out = nc.dram_tensor("out", [ROWS, W], dt.int32, kind="ExternalOutput")
    stages = {
        name: nc.dram_tensor(f"stage_{name}", [ROWS, cw], dt.int32, kind="Internal")
        for name, _, _, cw in SPLITS
    }

    zsrc = zcol.rearrange("(p n) one -> p (n one)", p=P)

    from contextlib import ExitStack

    es = ExitStack()
    with es:
        zv = es.enter_context(nc.sbuf_tensor("zv", [P, NQ], dt.float32))
        seeds2 = es.enter_context(nc.sbuf_tensor("seeds2", [P, NQ * CW], dt.int32))
        seedsP = es.enter_context(nc.sbuf_tensor("seedsP", [P, NQ * PW], dt.int32))
        in_sem = es.enter_context(nc.semaphore("in_sem"))
        cmp_sem = es.enter_context(nc.semaphore("cmp_sem"))
        sems = {
            name: (
                es.enter_context(nc.semaphore(f"st_{name}")),
                es.enter_context(nc.semaphore(f"dd_{name}")),
            )
            for name, _, _, _ in SPLITS
        }
        block = es.enter_context(nc.Block())

        def chain(eng, name, col0, ncols, cw, seeds_t, wait_cmp=True):
            st, dd = sems[name]
            stage = stages[name]
            if wait_cmp:
                eng.wait_ge(cmp_sem, 1)
            # stage[p*16+n, c] = seeds_t[p, (n c)]
            eng.dma_start(
                out=stage.rearrange("(p n) c -> p (n c)", p=P),
                in_=seeds_t[:, :],
            ).then_inc(st, 16)
            eng.wait_ge(st, 16)  # staging landed (same engine)
            # out[r, col0 + j*cw + c] = stage[r, c]
            nj = ncols // cw
            d2d_out = out[:, col0 : col0 + ncols].rearrange(
                "r (j c) -> r j c", c=cw
            )
            d2d_in = stage[:, :].unsqueeze(1).broadcast_to([ROWS, nj, cw])
            eng.dma_start(out=d2d_out, in_=d2d_in).then_inc(dd, 16)
            eng.wait_ge(dd, 16)  # output slice landed before kernel end

        def affine(out_ap, width):
            # out[p, n, c] = int32((zv[p, n] + 1e5) * 500), broadcast over c
            return nc.gpsimd.tensor_scalar(
                out_ap.rearrange("p (n c) -> p n c", c=width),
                zv[:, :].unsqueeze(-1).broadcast_to([P, NQ, width]),
                100000.0,
                500.0,
                alu.add,
                alu.mult,
            )

        @block.gpsimd
        def _(g):
            g.dma_start(out=zv[:, :], in_=zsrc).then_inc(in_sem, 16)
            g.wait_ge(in_sem, 16)  # true DMA completion (same engine)
            affine(seeds2[:, :], CW).then_inc(cmp_sem, 1)
            affine(seedsP[:, :], PW)
            # same engine: seedsP ready in program order
            chain(g, "gpsimd", 2 * HALF, POOL_COLS, PW, seedsP, wait_cmp=False)

        @block.sync
        def _(sync):
            chain(sync, "sync", 0, HALF, CW, seeds2)

        @block.scalar
        def _(scalar):
            chain(scalar, "scalar", HALF, HALF, CW, seeds2)

    _nc_cache["nc"] = nc
    return nc


def kernel(z, c=None, **_unused):
    z = np.ascontiguousarray(np.asarray(z), dtype=np.float32)
    assert z.shape == (B, W), z.shape
    nc = build_nc()
    from concourse.bass_utils import run_bass_kernel_spmd

    in_maps = []
    for i in range(N_CORES):
        zc = np.ascontiguousarray(z[i * ROWS : (i + 1) * ROWS, 0:1])
        in_maps.append({"zcol": zc})
    res = run_bass_kernel_spmd(nc, in_maps, core_ids=list(range(N_CORES)))
    globals()["LAST_RESULT"] = res
    return np.concatenate([r["out"] for r in res.results], axis=0).astype(np.int32)


# revision 12
# speedup vs baseline: 2.3778x; 1.0010x over previous
"""Trainium2 Bass kernel for nn_MappingNetwork (histogram_binning).

reference: seeds = searchsorted(linspace(-1e5, 1e5, 1e8, f32), z[:, 0], 'left');
           out = broadcast(seeds[:, None], (16384, 512)).astype(int32)

The buckets are uniform with spacing 2e5/(1e8-1) ~= 0.002, so the searchsorted
index admits the closed-form affine  seed = (z + 1e5) * 500  (the spec's
sharding_hint itself suggests eliminating the bucket buffer via this closed
form).  Computed in f32 it sits within 6 indices of the bit-exact searchsorted
result — validated against an exact f64-fma emulation of the XLA-CPU linspace
over the full data range.  Relative error ~1.2e-7 against seeds of magnitude
5e7, five orders of magnitude inside the 2e-2 gate.

Per core (2048 rows, row r = p*16 + n -> zv[p, n]):
  1. the Pool engine loads the z[:, 0] shard (8KB) into [128, 16] SBUF via
     its software-DGE queue, waits on its own DMA-completion semaphore, and
     computes the seeds itself: one Pool-engine tensor_scalar producing
     width-2 seed pairs (signalled to sync/scalar immediately) and a second
     producing width-8 runs for its own slice — a single-engine
     load->compute chain with no cross-engine DMA-semaphore latency,
  2. all three DMA-capable engines INDEPENDENTLY stage their seed runs
     (16KB / 64KB) to their own internal DRAM scratch and wait on their own
     completion semaphores,
  3. three concurrent DRAM->DRAM broadcast DMAs fan each row's seed run out
     into the 4MB output shard (sync: columns 0-221, scalar: 222-441, Pool:
     442-511 — the Pool slice maximized under the software-DGE
     16K-descriptor ring limit at repeat count 7): src keeps the stride-0
     repeat dim in the middle with a contiguous innermost run (DGE-legal),
     dst walks [row, repeat, run] with every ISA dim count under 2^16.
Every engine holds the end-of-block barrier on its own DMA-completion
semaphores, so the kernel never signals done before the output is in DRAM.

Sharding: batch 16384 -> 8 cores x 2048 rows.
"""

import numpy as np

N_CORES = 8
B = 16384
W = 512
ROWS = B // N_CORES  # 2048 rows per core
P = 128
NQ = ROWS // P  # 16 queries per partition
CW = 2  # sync/scalar staged run width (seed pair)
PW = 10  # Pool staged run width
POOL_COLS = 70  # Pool d2d slice: repeat count 7 -> 2048*7 descs < 16384
HALF = (W - POOL_COLS) // 2  # 221 -> use 222/220 split below
SP_COLS = 222
ACT_COLS = W - POOL_COLS - SP_COLS  # 220

# (engine, first column, n columns, run width)
SPLITS = [
    ("sync", 0, SP_COLS, CW),
    ("scalar", SP_COLS, ACT_COLS, CW),
    ("gpsimd", SP_COLS + ACT_COLS, POOL_COLS, PW),
]

_nc_cache = {}


def build_nc():
    if "nc" in _nc_cache:
        return _nc_cache["nc"]
    import concourse.bass as bass
    import concourse.mybir as mybir

    dt = mybir.dt
    alu = mybir.AluOpType

    nc = bass.Bass(detect_race_conditions=False)
    zcol = nc.dram_tensor("zcol", [ROWS, 1], dt.float32, kind="ExternalInput")
    out = nc.dram_tensor("out", [ROWS, W], dt.int32, kind="ExternalOutput")
    stages = {
        name: nc.dram_tensor(f"stage_{name}", [ROWS, cw], dt.int32, kind="Internal")
        for name, _, _, cw in SPLITS
    }

    zsrc = zcol.rearrange("(p n) one -> p (n one)", p=P)

    from contextlib import ExitStack

    es = ExitStack()
    with es:
        zv = es.enter_context(nc.sbuf_tensor("zv", [P, NQ], dt.float32))
        seeds2 = es.enter_context(nc.sbuf_tensor("seeds2", [P, NQ * CW], dt.int32))
        seedsP = es.enter_context(nc.sbuf_tensor("seedsP", [P, NQ * PW], dt.int32))
        in_sem = es.enter_context(nc.semaphore("in_sem"))
        cmp_sem = es.enter_context(nc.semaphore("cmp_sem"))
        sems = {
            name: (
                es.enter_context(nc.semaphore(f"st_{name}")),
                es.enter_context(nc.semaphore(f"dd_{name}")),
            )
            for name, _, _, _ in SPLITS
        }
        block = es.enter_context(nc.Block())

        def chain(eng, name, col0, ncols, cw, seeds_t, wait_cmp=True):
            st, dd = sems[name]
            stage = stages[name]
            if wait_cmp:
                eng.wait_ge(cmp_sem, 1)
            # stage[p*16+n, c] = seeds_t[p, (n c)]
            eng.dma_start(
                out=stage.rearrange("(p n) c -> p (n c)", p=P),
                in_=seeds_t[:, :],
            ).then_inc(st, 16)
            eng.wait_ge(st, 16)  # staging landed (same engine)
            # out[r, col0 + j*cw + c] = stage[r, c]
            nj = ncols // cw
            d2d_out = out[:, col0 : col0 + ncols].rearrange(
                "r (j c) -> r j c", c=cw
            )
            d2d_in = stage[:, :].unsqueeze(1).broadcast_to([ROWS, nj, cw])
            eng.dma_start(out=d2d_out, in_=d2d_in).then_inc(dd, 16)
            eng.wait_ge(dd, 16)  # output slice landed before kernel end

        def affine(out_ap, width):
            # out[p, n, c] = int32((zv[p, n] + 1e5) * 500), broadcast over c
            return nc.gpsimd.tensor_scalar(
                out_ap.rearrange("p (n c) -> p n c", c=width),
                zv[:, :].unsqueeze(-1).broadcast_to([P, NQ, width]),
                100000.0,
                500.0,
                alu.add,
                alu.mult,
            )

        @block.gpsimd
        def _(g):
            g.dma_start(out=zv[:, :], in_=zsrc).then_inc(in_sem, 16)
            g.wait_ge(in_sem, 16)  # true DMA completion (same engine)
            affine(seeds2[:, :], CW).then_inc(cmp_sem, 1)
            affine(seedsP[:, :], PW)
            # same engine: seedsP ready in program order
            chain(g, "gpsimd", SP_COLS + ACT_COLS, POOL_COLS, PW, seedsP, wait_cmp=False)

        @block.sync
        def _(sync):
            chain(sync, "sync", 0, SP_COLS, CW, seeds2)

        @block.scalar
        def _(scalar):
            chain(scalar, "scalar", SP_COLS, ACT_COLS, CW, seeds2)

    _nc_cache["nc"] = nc
    return nc


def kernel(z, c=None, **_unused):
    z = np.ascontiguousarray(np.asarray(z), dtype=np.float32)
    assert z.shape == (B, W), z.shape
    nc = build_nc()
    from concourse.bass_utils import run_bass_kernel_spmd

    in_maps = []
    for i in range(N_CORES):
        zc = np.ascontiguousarray(z[i * ROWS : (i + 1) * ROWS, 0:1])
        in_maps.append({"zcol": zc})
    res = run_bass_kernel_spmd(nc, in_maps, core_ids=list(range(N_CORES)))
    globals()["LAST_RESULT"] = res
    return np.concatenate([r["out"] for r in res.results], axis=0).astype(np.int32)
